# revision 1
# baseline (speedup 1.0000x reference)
"""DiffGLCM Trainium2 kernel.

Reference: t_j = A_j - A_{j+1} per pixel with A = [1, sigma_1..sigma_63, 0],
GLCM = sum_p t_c(p) outer t_p(p), normalized per image.

Kernel computes S = sum_p A_c(p) outer A_p(p) (65x65, raw sigmoid
co-occurrence) on the PE; the difference is linear, so on host
G[i,j] = S[i,j] - S[i+1,j] - S[i,j+1] + S[i+1,j+1]. The whole A vector
comes from ONE activation op: row 0 = sigmoid(640(x+10)) == 1 exactly,
rows 1..63 = bin edges, row 64 = sigmoid(640(x-11)) == 0 exactly.
Per-pixel-chunk matmuls (K<=128 pixel rows, M=N=65) accumulate in fp32
PSUM, split into 4 accumulators (2 strips x 2 parity groups) to reduce
fp32 accumulation error of the large raw sums; host sums them in fp64.
Batch of 16 images -> 2 per NeuronCore x 8 cores.
"""

import sys

sys.path.insert(0, "/opt/trn_rl_repo")

import numpy as np

import concourse.bass as bass
import concourse.mybir as mybir
import concourse.tile as tile
from concourse.bass_utils import run_bass_kernel_spmd

F32 = mybir.dt.float32
BF16 = mybir.dt.bfloat16
H = W = 256
NIMG = 2          # images per core
NG = 64           # grey levels
NR = NG + 1       # A rows: const-1, 63 edges, const-0
COLS = 255        # center/periph columns per strip
# (center row offset, periph row offset, rows) — periph = center + (1,1)
STRIPS = [(0, 1, 128), (128, 129, 127)]
COL_BATCHES = [(0, 64), (64, 64), (128, 64), (192, 63)]
N_ACC = 4         # PSUM accumulators per image


def _build_program(split=True, mm_dtype=BF16, loop_reps=0):
    import contextlib

    nc = bass.Bass()
    xs = nc.declare_dram_parameter("xs", [NIMG, H, W], F32, isOutput=False)
    shift = nc.declare_dram_parameter("shift", [128, NR * 64], F32, isOutput=False)
    out = nc.declare_dram_parameter("glcm", [NIMG, N_ACC, NR, NR], F32, isOutput=True)

    with tile.TileContext(nc) as tc:
        with (
            tc.tile_pool(name="const", bufs=1) as const_pool,
            tc.tile_pool(name="strips", bufs=2) as strip_pool,
            tc.tile_pool(name="arg", bufs=3) as arg_pool,
            tc.tile_pool(name="sig", bufs=4) as sig_pool,
            tc.tile_pool(name="oub", bufs=2) as out_pool,
            tc.tile_pool(name="ps", bufs=2, space="PSUM") as psum_pool,
        ):
            # shift replicated over columns; absorbed into SBUF via one
            # copy so downstream consumers never wait on the DMA queues.
            shift_raw = const_pool.tile([128, NR, 64], F32)
            nc.sync.dma_start(
                shift_raw[:].rearrange("p a b -> p (a b)"), shift[:]
            )
            sh2 = const_pool.tile([128, NR, 64], F32)
            nc.vector.tensor_copy(
                sh2[:].rearrange("p a b -> p (a b)"),
                shift_raw[:].rearrange("p a b -> p (a b)"),
            )

            rep_ctx = (
                tc.For_i(0, loop_reps, 1) if loop_reps else contextlib.nullcontext()
            )
            with rep_ctx:
              for img in range(NIMG):
                psums = []
                for g in range(N_ACC):
                    pst = psum_pool.tile([NR, NR], F32, tag=f"ps{g}", name=f"ps{g}")
                    psums.append(pst)
                # matmuls per accumulator: strip s parity q -> 255 cols split
                acc_mm = [0] * N_ACC
                acc_total = [128, 127, 128, 127]  # ceil/floor of 255 by parity

                for si, (r0c, r0p, P) in enumerate(STRIPS):
                    svc = {}
                    for nm, r0, cc in (("c", r0c, 0), ("p", r0p, 1)):
                        s = strip_pool.tile(
                            [128, COLS], F32, tag="s" + nm, name="s" + nm
                        )
                        nc.sync.dma_start(s[:P], xs[img, r0 : r0 + P, cc : cc + COLS])
                        s2 = strip_pool.tile(
                            [128, COLS], F32, tag="s2" + nm, name="s2" + nm
                        )
                        nc.vector.tensor_copy(s2[:P], s[:P])
                        svc[nm] = s2
                    for bi, (c0, CB) in enumerate(COL_BATCHES):
                        sgs = {}
                        for nm in ("c", "p"):
                            A = arg_pool.tile([128, NR, 64], F32, tag="arg", name="A")
                            xb = (
                                svc[nm][:P, c0 : c0 + CB]
                                .unsqueeze(1)
                                .broadcast_to([P, NR, CB])
                            )
                            # A = x - shift: row 0 -> x+10, row 64 -> x-11
                            shb = (
                                sh2[:P, :, 0]
                                .unsqueeze(2)
                                .broadcast_to([P, NR, CB])
                            )
                            sub_eng = nc.vector if (bi % 2 == 0) == (nm == 'c') else nc.gpsimd
                            sub_eng.tensor_sub(A[:P, :, 0:CB], xb, shb)
                            # sig = sigmoid(640*A); rows 0/64 exactly 1/0
                            sg = sig_pool.tile(
                                [128, NR, 64], mm_dtype, tag="sg" + nm, name="sg" + nm
                            )
                            nc.scalar.activation(
                                sg[:P, :, 0:CB],
                                A[:P, :, 0:CB],
                                mybir.ActivationFunctionType.Sigmoid,
                                scale=640.0,
                            )
                            sgs[nm] = sg
                        for c in range(CB):
                            acc = 2 * si + ((c0 + c) % 2)
                            nc.tensor.matmul(
                                psums[acc][:, :],
                                sgs["c"][:P, :, c],
                                sgs["p"][:P, :, c],
                                start=(acc_mm[acc] == 0),
                                stop=(acc_mm[acc] == acc_total[acc] - 1),
                            )
                            acc_mm[acc] += 1
                # ob: [65 partitions, N_ACC, 65] — each psum copied to one slot
                ob = out_pool.tile([NR, N_ACC, NR], F32, name="ob")
                for g in range(N_ACC):
                    nc.vector.tensor_copy(ob[:, g, :], psums[g][:, :])
                nc.sync.dma_start(
                    out[img].rearrange("a r c -> r a c"), ob[:]
                )
    if split:
        _split_waits(nc)
    return nc


def _split_waits(nc):
    """This walrus build rejects >1 sync wait on ANY instruction struct
    (even Tile's own end-of-kernel drain). Rewrite every multi-wait
    instruction into a chain of single-wait same-engine drains followed
    by the instruction carrying its last wait.
    """
    n = 0
    for bb in nc.m.functions[0].blocks:
        out = []
        for ins in bb.instructions:
            si = ins.sync_info
            if si is not None and si.on_wait and len(si.on_wait) > 1:
                waits = list(si.on_wait)
                for w in waits[:-1]:
                    out.append(
                        mybir.InstDrain(
                            name=f"waitsplit-{n}",
                            engine=ins.engine,
                            sync_info=mybir.SyncInfo(on_wait=[w], on_update=[]),
                        )
                    )
                    n += 1
                ins.sync_info = mybir.SyncInfo(
                    on_wait=waits[-1:], on_update=list(si.on_update or [])
                )
            out.append(ins)
        bb.instructions[:] = out
    return n


def make_in_maps(x):
    # shift[0] = -10 (sigmoid == 1), shift[k] = k/64, shift[64] = +11
    # (sigmoid == 0 for x in [0,1))
    sv = np.arange(0, NR, dtype=np.float32) / np.float32(NG)
    sv[0] = -10.0
    sv[NG] = 11.0
    shift = np.ascontiguousarray(
        np.broadcast_to(np.repeat(sv, 64)[None, :], (128, NR * 64))
    )
    return [
        {"xs": np.ascontiguousarray(x[2 * k : 2 * k + 2]), "shift": shift}
        for k in range(8)
    ]


def _finish_host(raw):
    # raw: [16, N_ACC, NR, NR] — fp64-sum accumulators, 2D second
    # difference (the E transform on both axes), then normalize.
    s = raw.astype(np.float64).sum(axis=1)  # [16, NR, NR]
    g = s[:, :NG, :NG] - s[:, 1:, :NG] - s[:, :NG, 1:] + s[:, 1:, 1:]
    g = g / g.sum(axis=(1, 2), keepdims=True)
    return g.astype(np.float32)


_NC = None


def kernel(x, offset_r=1, offset_c=1, **_):
    global _NC
    assert int(offset_r) == 1 and int(offset_c) == 1
    x = np.ascontiguousarray(np.asarray(x, dtype=np.float32).reshape(16, H, W))
    if _NC is None:
        _NC = _build_program()
    res = run_bass_kernel_spmd(_NC, make_in_maps(x), core_ids=list(range(8)))
    raw = np.concatenate([r["glcm"] for r in res.results], axis=0)
    return _finish_host(raw).reshape(16, 1, NG, NG, 1)


if __name__ == "__main__":
    _build_program()
    print("build OK")



# revision 2
# speedup vs baseline: 1.0825x; 1.0825x over previous
"""DiffGLCM Trainium2 kernel, v5.

Reference: t_j = A_j - A_{j+1} per pixel with A_k = sigmoid(640x - 10k)
(A_0 == 1, A_64 == 0 for x in [0,1)); GLCM = sum_p t_c(p) outer t_p(p),
normalized per image. Kernel computes raw S = sum_p A_c(p) outer A_p(p)
on the PE; host applies the (linear) second difference and normalizes.

Structure:
- A_k = sigmoid(640*x - 10k): bias is a per-partition scalar AP ->
  ONE activation instruction per bin over a whole image, no tensor_sub.
  A is computed ONCE per image (compared to separate center/periph
  copies): half the activation work of the baseline and no gpsimd/DVE
  elementwise at all.
- Row-pair parity layout: partition p holds image rows 2p and 2p+1 in
  the free dim (AE[p, bin, j, col], j = row parity). Even row pairs
  (2p, 2p+1) pair partition-aligned operands with a free-dim j-shift.
  Odd pairs (2p+1, 2p+2) use AO[p] = AE[p+1, :, 0, :], one
  partition-shifted DVE copy per bin (cheap; DVE is otherwise idle).
- 4 PSUM accumulator chains (pair parity x column parity), host-summed
  in fp64 to control fp32 accumulation error, then second difference +
  normalize on host. Batch 16 -> 2 images per core x 8 cores.
"""

import sys

sys.path.insert(0, "/opt/trn_rl_repo")

import numpy as np

import concourse.bass as bass
import concourse.mybir as mybir
import concourse.tile as tile
from concourse.bass_utils import run_bass_kernel_spmd

F32 = mybir.dt.float32
BF16 = mybir.dt.bfloat16
H = W = 256
NIMG = 2          # images per core
NG = 64           # grey levels
NR = NG + 1       # A rows: const-1, 63 edges, const-0
N_ACC = 4         # PSUM accumulators per image
SIG = mybir.ActivationFunctionType.Sigmoid


def _build_program(split=True, loop_reps=0, unroll=1):
    import contextlib

    nc = bass.Bass()
    xs = nc.declare_dram_parameter("xs", [NIMG, H, W], F32, isOutput=False)
    shiftb = nc.declare_dram_parameter("shiftb", [128, NR], F32, isOutput=False)
    out = nc.declare_dram_parameter("glcm", [NIMG, N_ACC, NR, NR], F32, isOutput=True)

    with tile.TileContext(nc) as tc:
        with (
            tc.tile_pool(name="const", bufs=1) as const_pool,
            tc.tile_pool(name="xp", bufs=2) as x_pool,
            tc.tile_pool(name="ae", bufs=2) as ae_pool,
            tc.tile_pool(name="ao", bufs=2) as ao_pool,
            tc.tile_pool(name="oub", bufs=2) as out_pool,
            tc.tile_pool(name="ps", bufs=2, space="PSUM") as psum_pool,
        ):
            # per-bin activation bias: shbm[:, k] = -640*shift[k] = -10k
            shb = const_pool.tile([128, NR], F32)
            nc.sync.dma_start(shb[:], shiftb[:])
            shbm = const_pool.tile([128, NR], F32)
            nc.vector.tensor_scalar_mul(shbm[:], shb[:], -640.0)

            rep_ctx = (
                tc.For_i(0, loop_reps, 1) if loop_reps else contextlib.nullcontext()
            )

            def produce_A(img):
                """DMA x, compute AE (63 activations), AO (shuffles + patch)."""
                xe = x_pool.tile([128, 2, 256], F32, tag="x", name="xe")
                nc.sync.dma_start(
                    xe[:].rearrange("p j c -> p (j c)"),
                    xs[img].rearrange("(p j) c -> p (j c)", j=2),
                )
                AE = ae_pool.tile([128, NR, 2, 256], BF16, tag="AE", name="AE")
                AO = ao_pool.tile([128, NR, 256], BF16, tag="AO", name="AO")
                nc.vector.memset(AE[:, 0, :, :], 1.0)
                nc.vector.memset(AE[:, NG, :, :], 0.0)
                nc.vector.memset(AO[:, 0, :], 1.0)
                nc.vector.memset(AO[:, NG, :], 0.0)
                shuf1 = [min(i + 1, 31) for i in range(32)]
                for k in range(1, NG):
                    nc.scalar.activation(
                        AE[:, k, :, :], xe[:, :, :], SIG,
                        scale=640.0, bias=shbm[:, k : k + 1],
                    )
                    nc.vector.stream_shuffle(AO[:, k, :], AE[:, k, 0, :], shuf1)
                for src_p in (32, 64, 96):
                    nc.sync.dma_start(
                        AO[src_p - 1 : src_p, :, :],
                        AE[src_p : src_p + 1, :, 0, :],
                    )
                return AE, AO

            def matmul_out(img, AE, AO):
                """Co-occurrence matmuls (4 contiguous-column PSUM chains),
                PSUM -> SBUF copies, output DMA."""
                psums = []
                for g in range(N_ACC):
                    pst = psum_pool.tile([NR, NR], F32, tag=f"ps{g}", name=f"ps{g}")
                    psums.append(pst)
                acc_mm = [0] * N_ACC
                acc_total = [128, 127, 128, 127]
                for c in range(255):
                    par = c % 2
                    g = par
                    nc.tensor.matmul(
                        psums[g][:, :],
                        AE[:, :, 0, c],
                        AE[:, :, 1, c + 1],
                        start=(acc_mm[g] == 0),
                        stop=(acc_mm[g] == acc_total[g] - 1),
                    )
                    acc_mm[g] += 1
                    g = 2 + par
                    nc.tensor.matmul(
                        psums[g][:, :],
                        AE[0:127, :, 1, c],
                        AO[0:127, :, c + 1],
                        start=(acc_mm[g] == 0),
                        stop=(acc_mm[g] == acc_total[g] - 1),
                    )
                    acc_mm[g] += 1
                ob = out_pool.tile([NR, N_ACC, NR], F32, tag="ob", name="ob")
                for g in range(N_ACC):
                    nc.vector.tensor_copy(ob[:, g, :], psums[g][:, :])
                nc.sync.dma_start(out[img].rearrange("a r c -> r a c"), ob[:])

            with rep_ctx:
                imgs = [i % NIMG for i in range(NIMG * unroll)]
                # software pipeline: produce A one image ahead of the matmuls
                pend = []
                for i, img in enumerate(imgs):
                    pend.append((img, produce_A(img)))
                    if i >= 1:
                        pimg, (AE, AO) = pend.pop(0)
                        matmul_out(pimg, AE, AO)
                for pimg, (AE, AO) in pend:
                    matmul_out(pimg, AE, AO)
    if split:
        _split_waits(nc)
    return nc


def _split_waits(nc):
    """This walrus build rejects >1 sync wait on ANY instruction struct.
    Rewrite every multi-wait instruction into a chain of single-wait
    same-engine drains followed by the instruction carrying its last wait.
    """
    n = 0
    for bb in nc.m.functions[0].blocks:
        out = []
        for ins in bb.instructions:
            si = ins.sync_info
            if si is not None and si.on_wait and len(si.on_wait) > 1:
                waits = list(si.on_wait)
                for w in waits[:-1]:
                    out.append(
                        mybir.InstDrain(
                            name=f"waitsplit-{n}",
                            engine=ins.engine,
                            sync_info=mybir.SyncInfo(on_wait=[w], on_update=[]),
                        )
                    )
                    n += 1
                ins.sync_info = mybir.SyncInfo(
                    on_wait=waits[-1:], on_update=list(si.on_update or [])
                )
            out.append(ins)
        bb.instructions[:] = out
    return n


def make_in_maps(x):
    # shiftb free-dim values: shift[0] = -10 (sigmoid == 1),
    # shift[k] = k/64, shift[64] = +11 (sigmoid == 0 for x in [0,1)).
    sv = np.arange(0, NR, dtype=np.float32) / np.float32(NG)
    sv[0] = -10.0
    sv[NG] = 11.0
    shiftb = np.ascontiguousarray(np.broadcast_to(sv[None, :], (128, NR)))
    return [
        {"xs": np.ascontiguousarray(x[2 * k : 2 * k + 2]), "shiftb": shiftb}
        for k in range(8)
    ]


def _finish_host(raw):
    # raw: [16, N_ACC, NR, NR] — fp64-sum accumulators, 2D second
    # difference, then normalize.
    s = raw.astype(np.float64).sum(axis=1)  # [16, NR, NR]
    g = s[:, :NG, :NG] - s[:, 1:, :NG] - s[:, :NG, 1:] + s[:, 1:, 1:]
    g = g / g.sum(axis=(1, 2), keepdims=True)
    return g.astype(np.float32)


_NC = None


def kernel(x, offset_r=1, offset_c=1, **_):
    global _NC
    assert int(offset_r) == 1 and int(offset_c) == 1
    x = np.ascontiguousarray(np.asarray(x, dtype=np.float32).reshape(16, H, W))
    if _NC is None:
        _NC = _build_program()
    res = run_bass_kernel_spmd(_NC, make_in_maps(x), core_ids=list(range(8)))
    raw = np.concatenate([r["glcm"] for r in res.results], axis=0)
    return _finish_host(raw).reshape(16, 1, NG, NG, 1)


if __name__ == "__main__":
    _build_program()
    print("build OK")


# revision 3
# speedup vs baseline: 1.8008x; 1.6635x over previous
"""DiffGLCM Trainium2 kernel, v9 (fused even/odd matmuls, M=128).

Reference: t_j = A_j - A_{j+1} per pixel with A_k = sigmoid(640x - 10k)
(A_0 == 1, A_64 == 0 for x in [0,1)); GLCM = sum_p t_c(p) outer t_p(p),
normalized per image. Kernel computes raw S = sum_p A_c(p) outer A_p(p)
on the PE; host applies the (linear) second difference and normalizes.

Layout: partition p holds image rows 2p (slot 0) and 2p+1 (slot 1);
slot 2 holds rows 2p-1 shifted down one partition AND pre-shifted by
+2 columns (AOs[:, c] = A(2p-1, c-2)), so one stationary access pattern
can address slot1 @ col c+1 and slot2 @ col c+1 (= odd center @ c-1).

Fused matmul m_c (c = 1..254), one PE instruction for TWO row-pair
groups:
  moving  = slot0 @ col c          (A(2p, c): even center / odd periph)
  stationary M=128 = [slot1 @ c+1 bins 0..63 | slot2 @ c+1 bins 0..63]
  out rows 0..63  += S_even(c)^T   (periph x center), bin-64 row == 0
  out rows 64..127 += S_odd(c-1)   (center x periph), bin-64 row == 0
plus one even-only (c=0) and one odd-only (c'=254) M=64 matmul.
256 matmuls per image instead of 510 (PE is instruction-count-bound at
~115-160 ns/instruction on TRN2).

A_k = sigmoid(640x - 10k) via one activation per bin (bias = per-
partition scalar AP), computed once per image. 4 PSUM accumulator
chains (column mod 4), host-summed in fp64. 16 images -> 2 per core.
"""

import sys

sys.path.insert(0, "/opt/trn_rl_repo")

import numpy as np

import concourse.bass as bass
import concourse.mybir as mybir
import concourse.tile as tile
from concourse.bass_utils import run_bass_kernel_spmd

F32 = mybir.dt.float32
BF16 = mybir.dt.bfloat16
H = W = 256
NIMG = 2          # images per core
NG = 64           # grey levels
NR = NG + 1       # A rows: const-1, 63 edges, const-0
N_ACC = 4         # PSUM accumulators per image
TC = 258          # tile column width (256 + 2 for the slot-2 col shift)
SIG = mybir.ActivationFunctionType.Sigmoid


def _build_program(split=True, loop_reps=0):
    import contextlib

    nc = bass.Bass()
    xs = nc.declare_dram_parameter("xs", [NIMG, H, W], F32, isOutput=False)
    shiftb = nc.declare_dram_parameter("shiftb", [128, NR], F32, isOutput=False)
    out = nc.declare_dram_parameter("glcm", [NIMG, N_ACC, 128, NG], F32, isOutput=True)

    with tile.TileContext(nc) as tc:
        with (
            tc.tile_pool(name="const", bufs=1) as const_pool,
            tc.tile_pool(name="xp", bufs=2) as x_pool,
            tc.tile_pool(name="tp", bufs=2) as t_pool,
            tc.tile_pool(name="oub", bufs=2) as out_pool,
            tc.tile_pool(name="ps", bufs=2, space="PSUM") as psum_pool,
        ):
            # per-bin activation bias: shbm[:, k] = -640*shift[k] = -10k
            shb = const_pool.tile([128, NR], F32)
            nc.sync.dma_start(shb[:], shiftb[:])
            shbm = const_pool.tile([128, NR], F32)
            nc.vector.tensor_scalar_mul(shbm[:], shb[:], -640.0)

            rep_ctx = (
                tc.For_i(0, loop_reps, 1) if loop_reps else contextlib.nullcontext()
            )

            def produce_T(img):
                """x DMA + activations + shifted copy into one 3-slot tile."""
                xe = x_pool.tile([128, 2, 256], F32, tag="x", name="xe")
                nc.sync.dma_start(
                    xe[:].rearrange("p j c -> p (j c)"),
                    xs[img].rearrange("(p j) c -> p (j c)", j=2),
                )
                # slot-major, 64 bins (bin 64 == 0 dropped entirely): the
                # fused stationary [slot1 bins | slot2 bins] then collapses
                # to ONE uniform-stride free dim of 128 (BIR requires it).
                T = t_pool.tile([128, 3, NG, TC], BF16, tag="T", name="T")
                nc.vector.memset(T[:, :, 0, :], 1.0)
                # shift-down-by-1 within 32-partition groups; slot2 is also
                # shifted +2 along columns so stationary APs share a column.
                shuf1 = [max(i - 1, 0) for i in range(32)]
                for k in range(1, NG):
                    nc.scalar.activation(
                        T[:, 0:2, k, 0:256], xe[:, :, :], SIG,
                        scale=640.0, bias=shbm[:, k : k + 1],
                    )
                    nc.vector.stream_shuffle(
                        T[:, 2, k, 2:TC], T[:, 1, k, 0:256], shuf1
                    )
                # bin-0 rows of slot2 (shuffle only writes k=1..63)
                nc.vector.stream_shuffle(T[:, 2, 0, 2:TC], T[:, 1, 0, 0:256], shuf1)
                # patch group-boundary partitions 32/64/96 (cross-group)
                for dst in (32, 64, 96):
                    nc.sync.dma_start(
                        T[dst : dst + 1, 2, :, 2:TC],
                        T[dst - 1 : dst, 1, :, 0:256],
                    )
                # odd pair at p=0 is (row -1, row 0): invalid -> zero center
                nc.vector.memset(T[0:1, 2, :, :], 0.0)
                # slot2 cols 0..1 are never shuffled; zero them so the c=0
                # fused matmul's S_odd(-1) half contributes exactly 0
                nc.vector.memset(T[:, 2, :, 0:2], 0.0)
                return T

            def matmul_out(img, T):
                psums = []
                for g in range(N_ACC):
                    pst = psum_pool.tile([128, NG], F32, tag=f"ps{g}", name=f"ps{g}")
                    psums.append(pst)
                acc_mm = [0] * N_ACC
                # chain lengths: fused c=0..254 by c%4, odd-only appended
                # to chain 3 (c=0's S_odd(-1) half is all-zero by constr.)
                acc_total = [64, 64, 64, 64]
                for c in range(0, 255):
                    g = c % N_ACC
                    nc.tensor.matmul(
                        psums[g][:, :],
                        T[:, 1:3, :, c + 1].rearrange("p j k -> p (j k)"),
                        T[:, 0, :, c],
                        start=(acc_mm[g] == 0),
                        stop=(acc_mm[g] == acc_total[g] - 1),
                    )
                    acc_mm[g] += 1
                # odd-only (c'=254): S_odd(254) into rows 64..127
                nc.tensor.matmul(
                    psums[3][64:128, :],
                    T[:, 2, :, 256],
                    T[:, 0, :, 255],
                    start=False,
                    stop=True,
                )
                acc_mm[3] += 1
                assert acc_mm == acc_total, acc_mm
                ob = out_pool.tile([128, N_ACC, NG], F32, tag="ob", name="ob")
                for g in range(N_ACC):
                    nc.vector.tensor_copy(ob[:, g, :], psums[g][:, :])
                nc.sync.dma_start(out[img].rearrange("a r c -> r a c"), ob[:])

            with rep_ctx:
                T0 = produce_T(0)
                T1 = produce_T(1)
                matmul_out(0, T0)
                matmul_out(1, T1)
    if split:
        _split_waits(nc)
    return nc


def _split_waits(nc):
    """This walrus build rejects >1 sync wait on ANY instruction struct.
    Rewrite every multi-wait instruction into a chain of single-wait
    same-engine drains followed by the instruction carrying its last wait.
    """
    n = 0
    for bb in nc.m.functions[0].blocks:
        out = []
        for ins in bb.instructions:
            si = ins.sync_info
            if si is not None and si.on_wait and len(si.on_wait) > 1:
                waits = list(si.on_wait)
                for w in waits[:-1]:
                    out.append(
                        mybir.InstDrain(
                            name=f"waitsplit-{n}",
                            engine=ins.engine,
                            sync_info=mybir.SyncInfo(on_wait=[w], on_update=[]),
                        )
                    )
                    n += 1
                ins.sync_info = mybir.SyncInfo(
                    on_wait=waits[-1:], on_update=list(si.on_update or [])
                )
            out.append(ins)
        bb.instructions[:] = out
    return n


def make_in_maps(x):
    # shiftb free-dim values: shift[0] = -10 (sigmoid == 1),
    # shift[k] = k/64, shift[64] = +11 (sigmoid == 0 for x in [0,1)).
    sv = np.arange(0, NR, dtype=np.float32) / np.float32(NG)
    sv[0] = -10.0
    sv[NG] = 11.0
    shiftb = np.ascontiguousarray(np.broadcast_to(sv[None, :], (128, NR)))
    return [
        {"xs": np.ascontiguousarray(x[2 * k : 2 * k + 2]), "shiftb": shiftb}
        for k in range(8)
    ]


def _finish_host(raw):
    # raw: [16, N_ACC, 128, NG] — rows 0..63: S_even^T[periph 0..63,
    # center 0..63]; rows 64..127: S_odd[center 0..63, periph 0..63].
    # Missing bin-64 rows/cols are exactly 0. fp64-sum accumulators,
    # rebuild S, 2D second difference, normalize.
    acc = raw.astype(np.float64).sum(axis=1)  # [16, 128, NG]
    s = np.zeros((raw.shape[0], NR, NR))
    s[:, :NG, :NG] += np.transpose(acc[:, 0:64, :], (0, 2, 1))  # S_even
    s[:, :NG, :NG] += acc[:, 64:128, :]                         # S_odd
    g = s[:, :NG, :NG] - s[:, 1:, :NG] - s[:, :NG, 1:] + s[:, 1:, 1:]
    g = g / g.sum(axis=(1, 2), keepdims=True)
    return g.astype(np.float32)


_NC = None


def kernel(x, offset_r=1, offset_c=1, **_):
    global _NC
    assert int(offset_r) == 1 and int(offset_c) == 1
    x = np.ascontiguousarray(np.asarray(x, dtype=np.float32).reshape(16, H, W))
    if _NC is None:
        _NC = _build_program()
    res = run_bass_kernel_spmd(_NC, make_in_maps(x), core_ids=list(range(8)))
    raw = np.concatenate([r["glcm"] for r in res.results], axis=0)
    return _finish_host(raw).reshape(16, 1, NG, NG, 1)


if __name__ == "__main__":
    _build_program()
    print("build OK")
